# revision 21
# baseline (speedup 1.0000x reference)
"""Grouped SwiGLU MoE MLP (16 experts) on 8 NeuronCores, expert-parallel.

Reference computation, per expert e over its contiguous token slice xi:
    out = (silu(xi @ w_gate[e].T) * (xi @ w_up[e].T)) @ w_down[e].T

Sharding: expert-parallel. Core c owns experts {2c, 2c+1}; the host hands it
the matching contiguous 2048-token slice of x (tokens are pre-sorted by
expert), so no device-side collectives are needed. Everything is handed to
the device feature-major (transposed on host) so the token axis is the
matmul moving/free dimension:

  gateT[f,t] = sum_h wgT[h,f] * xT[h,t]      (PE: lhsT=wgT tile, rhs=xT)
  hidT[f,t]  = silu(gateT) * upT             (ACT silu + DVE mul)
  outT[h,t]  = sum_f wdT[f,h] * hidT[f,t]    (PE: lhsT=wdT tile, rhs=hidT)

Inputs are quantized to bf16 on the host (rel err ~4.4e-3, budget 2e-2):
bf16 matmuls run at the same 1 row/cycle PE rate as fp32r but get the
fast-weight-load path (fp32 LDWEIGHTS at 224ns/128cols outruns the 213ns
matmul and throttles the pipeline), and halve DMA + SBUF footprint.
fp32 PSUM accumulation; output stored bf16 and upcast on the host.

Schedule notes (from perfetto traces):
- Steady state runs at the PE floor: 216 ns per 512-col matmul chain slot
  (512 cyc @ 2.4 GHz + NX issue), >99% dense.
- Warmup matmuls on zeroed tiles open the HAM clock-gate (1.2->2.4 GHz)
  during the DMA fill so real matmuls never run throttled.
- Each dma_start costs its HWDGE engine ~0.6us of serialized descriptor
  generation, so x arrives in 10 consolidated transfers; silus would
  otherwise queue behind them on the ACT ring and stall PSUM turnover.
- x for both experts loads upfront on the ACT ring; weights stream
  just-in-time on the SP ring; output stores ride ACT, with the final
  stores split across both rings to shorten the kernel tail.
"""

import numpy as np
import ml_dtypes

import concourse.bass as bass
import concourse.bacc as bacc
import concourse.mybir as mybir
from concourse import tile
from concourse.bass_utils import run_bass_kernel_spmd

E, T, H, F = 16, 16384, 1024, 2048
NCORES = 8
EPC = E // NCORES          # experts per core
TPE = T // E               # tokens per expert (uniform fast path)
P = 128                    # SBUF partitions
HT = H // P                # 8 h-tiles (contraction tiles for gate/up)
FT = F // P                # 16 f-tiles
HGS = H // P               # 8 output h-groups for down proj
NT = 512                   # matmul moving free dim (PSUM bank = 512 fp32)
TH = TPE // NT             # 2 t-halves
FG = 8                     # f-groups for gate/up weight streaming
FPG = FT // FG             # f-tiles per group = 2
FGW = F // FG              # f columns per group = 256
HP = 4                     # down-proj h-group pairs (2 hg per weight chunk)

_F32 = mybir.dt.float32
_BF16 = mybir.dt.bfloat16
_NPBF16 = ml_dtypes.bfloat16

_CACHE = {}

# Set by run for test harness introspection (exec_time_ns, profile).
LAST_RESULTS = None
TRACE = False
TRACE_KW = {}
# "silu" uses the native ScalarE Silu LUT; "sigmoid" decomposes it as
# gate*sigmoid(gate) for CoreSim, which lacks a Silu implementation.
ACT_MODE = "silu"


def _build_nc():
    nc = bacc.Bacc()
    xt_d = nc.dram_tensor("xt", [EPC, H, TPE], _BF16, kind="ExternalInput")
    wg_d = nc.dram_tensor("wg", [EPC, H, F], _BF16, kind="ExternalInput")
    wu_d = nc.dram_tensor("wu", [EPC, H, F], _BF16, kind="ExternalInput")
    wd_d = nc.dram_tensor("wd", [EPC, F, H], _BF16, kind="ExternalInput")
    out_d = nc.dram_tensor("outT", [EPC, H, TPE], _BF16, kind="ExternalOutput")

    with tile.TileContext(nc) as tc:
        with (
            tc.tile_pool(name="xp0", bufs=HT) as xp0,
            tc.tile_pool(name="xp1", bufs=2) as xp1,
            tc.tile_pool(name="w0p", bufs=HT) as w0p,
            tc.tile_pool(name="wgp", bufs=4) as wgp,
            tc.tile_pool(name="wup", bufs=4) as wup,
            tc.tile_pool(name="wdp", bufs=3) as wdp,
            tc.tile_pool(name="hid", bufs=FT + 1) as hidp,
            tc.tile_pool(name="tmp", bufs=3) as tmpp,
            tc.tile_pool(name="osb", bufs=4) as osbp,
            tc.tile_pool(name="ps", bufs=8, space=bass.MemorySpace.PSUM) as psp,
        ):
            # PE warmup: ~3.4us of matmuls on zeroed tiles issued while the
            # first real DMAs are in flight, so the HAM clock-gate opens
            # (1.2 -> 2.4 GHz needs ~3.4us sustained PE activity) right as
            # real matmuls start.
            wl = tmpp.tile([P, P], _BF16, tag="warml")
            wr = tmpp.tile([P, NT], _BF16, tag="warmr")
            nc.gpsimd.memset(wl[:], 0)
            nc.gpsimd.memset(wr[:], 0)
            wps = psp.tile([P, NT], _F32, tag="ps", name="warm_ps")
            for _ in range(8):
                nc.tensor.matmul(wps[:], wl[:], wr[:], start=True, stop=True)

            # Startup-critical loads.  Group 0 of expert 0 runs th-outer
            # (t0-half chains over all ht, then t1-half), so the PE only
            # needs HALF of x before running dense.  Its weights stream as
            # ht-pairs interleaved wg/wu on the SP ring, matching the chain
            # consumption order; x streams as [128, 512] th-halves on the
            # ACT ring, th0 halves first.
            wg_v0 = wg_d[0].rearrange("(a p) f -> p a f", p=P)
            wu_v0 = wu_d[0].rearrange("(a p) f -> p a f", p=P)
            fsl0 = slice(0, FGW)
            w0g, w0u = [], []
            for j in range(HT // 2):
                tg = w0p.tile([P, 2, FGW], _BF16, tag="w0")
                nc.sync.dma_start(tg[:], wg_v0[:, 2 * j:2 * j + 2, fsl0])
                w0g.append(tg)
                tu = w0p.tile([P, 2, FGW], _BF16, tag="w0")
                nc.sync.dma_start(tu[:], wu_v0[:, 2 * j:2 * j + 2, fsl0])
                w0u.append(tu)

            # Token activations on the ACT HWDGE ring, in as FEW dma_start
            # calls as possible: each DMA_DIRECT2D costs the issuing engine
            # ~0.6us of serialized descriptor generation, and silus queue
            # behind them.  Expert 0 arrives as ht-pair th-half tiles
            # (th0 halves first, matching the th-outer group-0 schedule);
            # expert 1 as two quarter-expert full-width tiles.  32 KiB/
            # partition total, resident all kernel.
            x0_v = xt_d[0].rearrange("(a p) t -> p a t", p=P)
            x1_v = xt_d[1].rearrange("(a p) t -> p a t", p=P)
            x0t = {}
            for th in range(TH):
                for j in range(HT // 2):
                    t = xp0.tile([P, 2, NT], _BF16, tag="x0")
                    nc.scalar.dma_start(
                        t[:], x0_v[:, 2 * j:2 * j + 2,
                                   th * NT:(th + 1) * NT])
                    x0t[th, j] = t
            x1t = []
            for j in range(2):
                t = xp1.tile([P, HT // 2, TPE], _BF16, tag="x1")
                nc.scalar.dma_start(t[:], x1_v[:, 4 * j:4 * j + 4, :])
                x1t.append(t)

            def xsl(el, ht, th):
                if el == 0:
                    return x0t[th, ht // 2][:, ht % 2, :]
                return x1t[ht // 4][:, ht % 4, th * NT:(th + 1) * NT]

            for el in range(EPC):
                # DRAM views with the h-tile index split out of the partition
                # axis: [128p, HT, F].
                wg_v = wg_d[el].rearrange("(a p) f -> p a f", p=P)
                wu_v = wu_d[el].rearrange("(a p) f -> p a f", p=P)

                hidden = [hidp.tile([P, TPE], _BF16, tag="hid",
                                    name=f"hid{el}_{i}") for i in range(FT)]

                for fgi in range(FG):
                    fsl = slice(fgi * FGW, (fgi + 1) * FGW)
                    if el == 0 and fgi == 0:
                        # Uses the startup ht-pair tiles loaded above.
                        wg_sl = lambda ht, c0, c1, w=w0g: (
                            w[ht // 2][:, ht % 2, c0:c1])
                        wu_sl = lambda ht, c0, c1, w=w0u: (
                            w[ht // 2][:, ht % 2, c0:c1])
                    else:
                        wgt = wgp.tile([P, HT, FGW], _BF16, tag="wg")
                        nc.sync.dma_start(wgt[:], wg_v[:, :, fsl])
                        wut = wup.tile([P, HT, FGW], _BF16, tag="wu")
                        nc.sync.dma_start(wut[:], wu_v[:, :, fsl])
                        wg_sl = lambda ht, c0, c1, w=wgt: w[:, ht, c0:c1]
                        wu_sl = lambda ht, c0, c1, w=wut: w[:, ht, c0:c1]

                    gate_ps, up_ps = {}, {}
                    for ftl in range(FPG):
                        for th in range(TH):
                            gate_ps[ftl, th] = psp.tile(
                                [P, NT], _F32, tag="ps", name="gu_ps")
                            up_ps[ftl, th] = psp.tile(
                                [P, NT], _F32, tag="ps", name="gu_ps")
                    if el == 0 and fgi == 0:
                        # th-outer: the th0 chains run over all ht using only
                        # th0 x-halves, so the PE goes dense after ~1 MiB of
                        # DMA instead of ~2.5 MiB.
                        for th in range(TH):
                            for ht in range(HT):
                                for wsl, store in ((wg_sl, gate_ps),
                                                   (wu_sl, up_ps)):
                                    for ftl in range(FPG):
                                        nc.tensor.matmul(
                                            store[ftl, th][:],
                                            wsl(ht, ftl * P, (ftl + 1) * P),
                                            xsl(el, ht, th),
                                            start=(ht == 0),
                                            stop=(ht == HT - 1),
                                        )
                    else:
                        for wsl, store in ((wg_sl, gate_ps), (wu_sl, up_ps)):
                            for ftl in range(FPG):
                                for ht in range(HT):
                                    lhsT = wsl(ht, ftl * P, (ftl + 1) * P)
                                    for th in range(TH):
                                        nc.tensor.matmul(
                                            store[ftl, th][:],
                                            lhsT,
                                            xsl(el, ht, th),
                                            start=(ht == 0),
                                            stop=(ht == HT - 1),
                                        )
                    for ftl in range(FPG):
                        ft = fgi * FPG + ftl
                        for th in range(TH):
                            tsl = slice(th * NT, (th + 1) * NT)
                            tmp = tmpp.tile([P, NT], _F32, tag="tmp")
                            if ACT_MODE == "silu":
                                nc.scalar.activation(
                                    tmp[:], gate_ps[ftl, th][:],
                                    mybir.ActivationFunctionType.Silu,
                                )
                            else:
                                nc.scalar.activation(
                                    tmp[:], gate_ps[ftl, th][:],
                                    mybir.ActivationFunctionType.Sigmoid,
                                )
                                nc.vector.tensor_mul(
                                    tmp[:], tmp[:], gate_ps[ftl, th][:]
                                )
                            nc.vector.tensor_mul(
                                hidden[ft][:, tsl], tmp[:], up_ps[ftl, th][:]
                            )

                # Down projection: outT[h,t] accumulating over all 16 f-tiles.
                # Weights come in hg-pairs ([128, 16, 256] bf16) so DRAM
                # segments stay at 512B.
                wd_v = wd_d[el].rearrange("(a p) h -> p a h", p=P)
                for hp in range(HP):
                    wdt = wdp.tile([P, FT, 2 * P], _BF16, tag="wd")
                    nc.sync.dma_start(
                        wdt[:], wd_v[:, :, hp * 2 * P:(hp + 1) * 2 * P])
                    for sub in range(2):
                        hg = 2 * hp + sub
                        hsl = slice(hg * P, (hg + 1) * P)
                        ops = [psp.tile([P, NT], _F32, tag="ps", name="dn_ps")
                               for _ in range(TH)]
                        last = (el == EPC - 1 and hp == HP - 1 and sub == 1)
                        if last:
                            # Sequential th-chains: th0's copy+store overlap
                            # th1's 16-matmul chain; the final th1 output is
                            # halved across BOTH HWDGE rings so the kernel
                            # tail is one copy + two parallel 64 KiB stores.
                            for th in range(TH):
                                for ft in range(FT):
                                    nc.tensor.matmul(
                                        ops[th][:],
                                        wdt[:, ft, sub * P:(sub + 1) * P],
                                        hidden[ft][:, th * NT:(th + 1) * NT],
                                        start=(ft == 0),
                                        stop=(ft == FT - 1),
                                    )
                                if th == 0:
                                    # Four slice-stores alternating rings:
                                    # keeps BOTH HWDGE queues busy through
                                    # th1's 3.4us chain so the final stores
                                    # don't pay a cold-ring drain (~3us
                                    # observed).
                                    qn = NT // 4
                                    for s in range(4):
                                        c0 = s * qn
                                        osb = osbp.tile(
                                            [P, qn], _BF16, tag="osbq")
                                        nc.vector.tensor_copy(
                                            osb[:], ops[0][:, c0:c0 + qn])
                                        eng = nc.sync if s % 2 == 0 else nc.scalar
                                        eng.dma_start(
                                            out_d[el, hsl, c0:c0 + qn],
                                            osb[:])
                                else:
                                    hn = NT // 2
                                    for s in range(2):
                                        c0 = NT + s * hn
                                        osb = osbp.tile(
                                            [P, hn], _BF16, tag="osbh")
                                        nc.vector.tensor_copy(
                                            osb[:],
                                            ops[1][:, s * hn:(s + 1) * hn])
                                        eng = nc.sync if s == 0 else nc.scalar
                                        eng.dma_start(
                                            out_d[el, hsl, c0:c0 + hn],
                                            osb[:])
                        else:
                            for ft in range(FT):
                                lhsT = wdt[:, ft, sub * P:(sub + 1) * P]
                                for th in range(TH):
                                    nc.tensor.matmul(
                                        ops[th][:],
                                        lhsT,
                                        hidden[ft][:, th * NT:(th + 1) * NT],
                                        start=(ft == 0),
                                        stop=(ft == FT - 1),
                                    )
                            # Per-th copies + stores; stores ride the ACT
                            # ring (x loads there finished long ago).
                            for th in range(TH):
                                tsl = slice(th * NT, (th + 1) * NT)
                                osb = osbp.tile([P, NT], _BF16, tag="osb")
                                nc.vector.tensor_copy(osb[:], ops[th][:])
                                nc.scalar.dma_start(
                                    out_d[el, hsl, tsl], osb[:])
    return nc


def get_nc():
    if "nc" not in _CACHE:
        nc = _build_nc()
        nc.finalize()
        _CACHE["nc"] = nc
    return _CACHE["nc"]


def make_in_maps(x, w_gate, w_up, w_down):
    in_maps = []
    for c in range(NCORES):
        e0 = c * EPC
        xs = x[e0 * TPE:(e0 + EPC) * TPE].reshape(EPC, TPE, H)
        in_maps.append({
            "xt": np.ascontiguousarray(
                xs.transpose(0, 2, 1)).astype(_NPBF16),
            "wg": np.ascontiguousarray(
                w_gate[e0:e0 + EPC].transpose(0, 2, 1)).astype(_NPBF16),
            "wu": np.ascontiguousarray(
                w_up[e0:e0 + EPC].transpose(0, 2, 1)).astype(_NPBF16),
            "wd": np.ascontiguousarray(
                w_down[e0:e0 + EPC].transpose(0, 2, 1)).astype(_NPBF16),
        })
    return in_maps


def _numpy_fallback(x, w_gate, w_up, w_down, counts):
    out = np.empty((x.shape[0], w_down.shape[1]), np.float32)
    o = 0
    for e in range(len(counts)):
        n = int(counts[e])
        xi = x[o:o + n]
        gate = xi @ w_gate[e].T
        up = xi @ w_up[e].T
        hidden = (gate / (1.0 + np.exp(-gate))) * up
        out[o:o + n] = hidden @ w_down[e].T
        o += n
    return out


def kernel(x, w_gate, w_up, w_down, tokens_per_expert):
    global LAST_RESULTS
    x = np.asarray(x, dtype=np.float32)
    w_gate = np.asarray(w_gate, dtype=np.float32)
    w_up = np.asarray(w_up, dtype=np.float32)
    w_down = np.asarray(w_down, dtype=np.float32)
    counts = np.asarray(tokens_per_expert).astype(np.int64)

    if not (counts.shape == (E,) and np.all(counts == TPE)):
        # Non-uniform routing: the compiled program is shaped for the
        # uniform split the reference generator produces.
        return _numpy_fallback(x, w_gate, w_up, w_down, counts)

    nc = get_nc()
    res = run_bass_kernel_spmd(
        nc, make_in_maps(x, w_gate, w_up, w_down), list(range(NCORES)),
        trace=TRACE, **TRACE_KW,
    )
    LAST_RESULTS = res
    out = np.empty((T, H), np.float32)
    for c in range(NCORES):
        o = res.results[c]["outT"].astype(np.float32)  # [EPC, H, TPE] bf16
        for el in range(EPC):
            t0 = (c * EPC + el) * TPE
            out[t0:t0 + TPE] = o[el].T
    return out
